# revision 1
# baseline (speedup 1.0000x reference)
"""Trainium2 Bass kernel for nn_MessagePassingLayer (GNN message passing).

reference semantics (per batch b):
  cm  = adj[b].T @ ps[b]                  # [C, H] channel aggregation
  ncs = GRUCell(x=cs[b], h=cm)            # new channel states
  pm  = adj[b] @ ncs                      # [P, H] path aggregation
  nps = GRUCell(x=ps[b], h=pm)            # new path states
  returns (nps, ncs)

Sharding: data-parallel over batch, 2 batches per core x 8 cores.

Per-core design (memory regime): the host pre-casts adj to fp16 and ships
BOTH adj [P, C] and adjT [C, P] — 2 x 8MB/batch, the same HBM bytes as one
f32 copy, so DMA traffic is unchanged while the kernel loses all on-device
transposition (PE-identity strip transposes + PSUM->SBUF copies of the
transposed tiles cost ~140us in the old design).

  - einsum1 streams adj p-slabs [128, C] (sync/HWDGE queue) as matmul
    moving data: cmT accumulates PACKED in one PSUM bank (h-rows 32n =
    c-chunk n via tile_position column packing).
  - einsum2 streams adjT c-slabs [128, P] (gpsimd queue, all 16 prefetched
    into SBUF): pmT accumulates over c-chunks with lhsT = ncs natural
    tiles, same packed single-bank layout.
  - GRU gates feature-major [32, N]: giT/ghT = wT-form matmuls; biases are
    per-partition ACT bias APs; combines are TensorTensor ops split
    half/half across DVE and Pool.
  - each batch's tail (GRU-c, einsum2, GRU-p, packing) is woven into the
    next batch's einsum1 slab window so PE/ACT/DVE overlap the DMA stream.
  - outputs packed on-chip to [q, (g l h)] (512B HBM runs), DMA'd on the
    vector queue so they never block the adj stream.
"""

import numpy as np

import concourse.bass as bass
import concourse.tile as tile
from concourse import bacc, masks, mybir
from concourse.bass_utils import run_bass_kernel_spmd

F32 = mybir.dt.float32
# fp16 (10-bit mantissa): adj in [0,1), states O(1), messages O(1e3) — all
# in range, 4x less rounding than bf16.
F16 = mybir.dt.float16
BF16 = F16

B, P, C, H = 16, 2048, 2048, 32
G = 3 * H  # 96
NCORES = 8
BPC = B // NCORES  # batches per core
PB = 128  # partition block
NP = P // PB  # 16 p-chunks
NC = C // PB  # 16 c-chunks
NKC = 512  # matmul moving chunk (one PSUM f32 bank)


def _gru_gen(tc, pool, ps_misc, wT_ih, wT_hh, xT, hT, b_rz, bias_n,
             st_pool, out_tag, holder, dt_b=BF16, follow=None):
    """Feature-major GRUCell -> SBUF [H, N] tile (dtype dt_b) in holder["out"].

    Generator: yields once per 512-col chunk. The full gate+combine chain
    runs per-chunk ([32, 512] ops) so downstream work (`follow(q)`) can
    start as soon as chunk q's outputs exist — chunk q+1's matmuls overlap
    chunk q's eltwise tail.

    Per chunk, one PSUM tile [128, 512] f32 holds:
      rows 0:64   = i_rz + h_rz   (two accumulating matmuls)
      rows 64:96  = i_n
      rows 96:128 = h_n
    """
    nc = tc.nc
    AF = mybir.ActivationFunctionType
    N = xT.shape[-1]
    out = st_pool.tile([H, N], dt_b, tag=out_tag, name="out")
    holder["out"] = out
    r = pool.tile([H, N], BF16, tag="gru_r", name="r")
    z = pool.tile([H, N], BF16, tag="gru_z", name="z")
    g = pool.tile([H, N], BF16, tag="gru_g", name="g")
    gin = pool.tile([H, N], BF16, tag="gru_gin", name="gin")
    t1 = pool.tile([H, N], BF16, tag="gru_t1", name="t1")
    ng = pool.tile([H, N], dt_b, tag="gru_ng", name="ng")
    d = pool.tile([H, N], dt_b, tag="gru_d", name="d")
    for q in range(N // NKC):
        gp = ps_misc.tile([PB, NKC], F32, tag="sm", name="gp")
        sl = slice(q * NKC, (q + 1) * NKC)
        nc.tensor.matmul(gp[0 : 2 * H, :], wT_ih[:, 0 : 2 * H], xT[:, sl],
                         start=True, stop=False)
        nc.tensor.matmul(gp[0 : 2 * H, :], wT_hh[:, 0 : 2 * H], hT[:, sl],
                         start=False, stop=True)
        nc.tensor.matmul(gp[2 * H : G, :], wT_ih[:, 2 * H : G], xT[:, sl],
                         start=True, stop=True)
        nc.tensor.matmul(gp[G : G + H, :], wT_hh[:, 2 * H : G], hT[:, sl],
                         start=True, stop=True, tile_position=(0, 96))
        nc.scalar.activation(r[:, sl], gp[0:H, :], AF.Sigmoid, bias=b_rz[0:H, :])
        nc.scalar.activation(z[:, sl], gp[H : 2 * H, :], AF.Sigmoid,
                             bias=b_rz[H : 2 * H, :])
        nc.vector.tensor_scalar_add(g[:, sl], gp[G : G + H, :],
                                    bias_n[G : G + H, :])
        nc.vector.tensor_scalar_add(gin[:, sl], gp[2 * H : G, :],
                                    bias_n[2 * H : G, :])
        nc.vector.tensor_mul(t1[:, sl], r[:, sl], g[:, sl])
        nc.vector.tensor_add(g[:, sl], gin[:, sl], t1[:, sl])
        nc.scalar.activation(ng[:, sl], g[:, sl], AF.Tanh)
        nc.vector.tensor_sub(d[:, sl], hT[:, sl], ng[:, sl])
        nc.vector.tensor_mul(t1[:, sl], z[:, sl], d[:, sl])
        nc.vector.tensor_add(out[:, sl], ng[:, sl], t1[:, sl])
        if follow is not None:
            follow(q)
        yield


def build_nc(debug_outputs=False, n_devices=NCORES):
    nc = bacc.Bacc("TRN2", target_bir_lowering=False, debug=False,
                   num_devices=n_devices)

    adj = nc.dram_tensor("adj16", [BPC, P, C], F16, kind="ExternalInput")
    adjT = nc.dram_tensor("adjT16", [BPC, C, P], F16, kind="ExternalInput")
    ps = nc.dram_tensor("ps", [BPC, P, H], F32, kind="ExternalInput")
    cs = nc.dram_tensor("cs", [BPC, C, H], F32, kind="ExternalInput")
    w_ih_c = nc.dram_tensor("w_ih_c", [G, H], F32, kind="ExternalInput")
    w_hh_c = nc.dram_tensor("w_hh_c", [G, H], F32, kind="ExternalInput")
    w_ih_p = nc.dram_tensor("w_ih_p", [G, H], F32, kind="ExternalInput")
    w_hh_p = nc.dram_tensor("w_hh_p", [G, H], F32, kind="ExternalInput")
    b_ih_c = nc.dram_tensor("b_ih_c", [G, 1], F32, kind="ExternalInput")
    b_hh_c = nc.dram_tensor("b_hh_c", [G, 1], F32, kind="ExternalInput")
    b_ih_p = nc.dram_tensor("b_ih_p", [G, 1], F32, kind="ExternalInput")
    b_hh_p = nc.dram_tensor("b_hh_p", [G, 1], F32, kind="ExternalInput")
    out_np = nc.dram_tensor("new_path", [BPC, P, H], F32, kind="ExternalOutput")
    out_nc = nc.dram_tensor("new_channel", [BPC, C, H], F32, kind="ExternalOutput")
    dbg = {}
    if debug_outputs:
        dbg["cmT"] = nc.dram_tensor("dbg_cmT", [BPC, H, C], F32, kind="ExternalOutput")
        dbg["pmT"] = nc.dram_tensor("dbg_pmT", [BPC, H, P], F32, kind="ExternalOutput")
        dbg["ncsT"] = nc.dram_tensor("dbg_ncsT", [BPC, H, C], F32, kind="ExternalOutput")

    with tile.TileContext(nc) as tc:
        _body(tc, adj, adjT, ps, cs,
              (w_ih_c, w_hh_c, b_ih_c, b_hh_c),
              (w_ih_p, w_hh_p, b_ih_p, b_hh_p),
              out_np, out_nc, dbg)
    nc.finalize()
    return nc


def _body(tc, adj, adjT, ps, cs, wc, wp, out_np, out_nc, dbg):
    nc = tc.nc
    from contextlib import ExitStack

    ctx = ExitStack()
    with ctx:
        const = ctx.enter_context(tc.tile_pool(name="const", bufs=1))
        a_pool = ctx.enter_context(tc.tile_pool(name="a_slabs", bufs=6))
        # all 16 adjT c-slabs of a batch stay resident (prefetch window)
        at_pool = ctx.enter_context(tc.tile_pool(name="at_slabs", bufs=16))
        st_pool = ctx.enter_context(tc.tile_pool(name="states", bufs=1))
        gru_pool = ctx.enter_context(tc.tile_pool(name="gru", bufs=1))
        out_pool = ctx.enter_context(tc.tile_pool(name="outs", bufs=2))
        # PSUM banks: ps_mm 2 (packed cm/pm, double-buffered) + ps_misc 3
        ps_mm = ctx.enter_context(tc.tile_pool(name="ps_mm", bufs=2, space="PSUM"))
        ps_misc = ctx.enter_context(tc.tile_pool(name="ps_misc", bufs=3, space="PSUM"))

        ident = const.tile([PB, PB], BF16)
        masks.make_identity(nc, ident[:])
        ident_f = const.tile([PB, PB], F32)
        masks.make_identity(nc, ident_f[:])
        idents = {BF16: ident, F32: ident_f}

        # ---- weights: load [G, H], transpose to [H, G] via identity matmul
        wT = {}
        for name, wdram, wdt in (("ihc", wc[0], BF16), ("hhc", wc[1], BF16),
                                 ("ihp", wp[0], BF16), ("hhp", wp[1], F32)):
            w_ld = const.tile([G, H], wdt, tag=f"w_{name}", name="w_ld")
            nc.gpsimd.dma_start(w_ld[:], wdram[:, :])
            wt_ps = ps_misc.tile([H, G], F32, tag="sm", name="wt_ps")
            nc.tensor.matmul(wt_ps[:], w_ld[:], idents[wdt][0:G, 0:G],
                             start=True, stop=True)
            wtile = const.tile([H, G], wdt, tag=f"wT_{name}", name="wtile")
            nc.scalar.copy(wtile[:], wt_ps[:])
            wT[name] = wtile

        # ---- biases ----
        bias = {}
        for s, (bih, bhh) in (("c", (wc[2], wc[3])), ("p", (wp[2], wp[3]))):
            bn = const.tile([PB, 1], F32, tag=f"bn_{s}", name="bn")
            nc.sync.dma_start(bn[2 * H : G, :], bih[2 * H : G, :])
            nc.sync.dma_start(bn[G : G + H, :], bhh[2 * H : G, :])
            ihrz = const.tile([2 * H, 1], F32, tag=f"bi_{s}", name="ihrz")
            nc.sync.dma_start(ihrz[:], bih[0 : 2 * H, :])
            hhrz = const.tile([2 * H, 1], F32, tag=f"bh_{s}", name="hhrz")
            nc.sync.dma_start(hhrz[:], bhh[0 : 2 * H, :])
            brz = const.tile([2 * H, 1], F32, tag=f"brz_{s}", name="brz")
            nc.vector.tensor_add(brz[:], ihrz[:], hhrz[:])
            bias[s] = (brz, bn)

        state = [dict() for _ in range(BPC)]

        def emit_states_dma(b):
            d = state[b]
            d["ps_nat"] = st_pool.tile([PB, NP, H], BF16, tag="ps_nat",
                                       name="ps_nat", bufs=2)
            nc.gpsimd.dma_start(
                d["ps_nat"][:], ps[b].rearrange("(i p) h -> p i h", p=PB))
            d["cs_nat"] = st_pool.tile([PB, NC, H], BF16, tag="cs_nat",
                                       name="cs_nat", bufs=2)
            nc.gpsimd.dma_start(
                d["cs_nat"][:], cs[b].rearrange("(i p) h -> p i h", p=PB))

        def emit_head(b):
            # feature-major states via PE quad transposes
            d = state[b]
            for nm, nat, nch in (("psT", d["ps_nat"], NP),
                                 ("csT", d["cs_nat"], NC)):
                dst = st_pool.tile([H, nch * PB], BF16, tag=nm, name="dst",
                                   bufs=2)
                for quad in range(nch // 4):
                    tp = ps_misc.tile([H, 4, PB], F32, tag="sm", name="tp")
                    for k in range(4):
                        nc.tensor.matmul(tp[:, k, :],
                                         nat[:, quad * 4 + k, :],
                                         ident[:, :], start=True, stop=True)
                    nc.scalar.copy(
                        dst[:, quad * 4 * PB : (quad + 1) * 4 * PB], tp[:])
                d[nm] = dst
            d["cmT"] = ps_mm.tile([PB, NKC], F32, tag="mm", name="cmT")

        def emit_slab(b, i):
            # einsum1: adj p-slab moving; cmT packed one bank (rows 32n).
            d = state[b]
            slab = a_pool.tile([PB, C], BF16, tag="a", name="slab")
            nc.sync.dma_start(slab[:], adj[b, i * PB : (i + 1) * PB, :])
            for n in range(C // NKC):
                nc.tensor.matmul(
                    d["cmT"][n * H : (n + 1) * H, :],
                    d["ps_nat"][:, i, :],
                    slab[:, n * NKC : (n + 1) * NKC],
                    start=(i == 0), stop=(i == NP - 1),
                    tile_position=(0, n * H), skip_group_check=True)

        def emit_at_prefetch(b):
            d = state[b]
            d["slabT"] = []
            for j in range(NC):
                slabT = at_pool.tile([PB, P], BF16, tag="at", name="slabT")
                nc.gpsimd.dma_start(slabT[:],
                                    adjT[b, j * PB : (j + 1) * PB, :])
                d["slabT"].append(slabT)

        def emit_cm_extract(b):
            d = state[b]
            cmT_s = st_pool.tile([H, C], BF16, tag="hback", name="cmT_s")
            for n in range(4):
                nc.scalar.copy(cmT_s[:, n * NKC : (n + 1) * NKC],
                               d["cmT"][n * H : (n + 1) * H, :])
            d["cmT_s"] = cmT_s
            if "cmT" in dbg:
                nc.scalar.dma_start(dbg["cmT"][b], cmT_s[:])

        def tail_gen(b):
            d = state[b]
            # ---- GRU-c -> ncs_nat -> einsum2, pipelined per 512-chunk ----
            pmT = ps_mm.tile([PB, NKC], F32, tag="mm", name="pmT")
            hold = {}

            def follow_c(q):
                ncsT = hold["out"]
                tpn = ps_misc.tile([PB, 4, H], F32, tag="sm", name="tpn")
                for k in range(4):
                    j = 4 * q + k
                    nc.tensor.matmul(tpn[:, k, :],
                                     ncsT[:, j * PB : (j + 1) * PB],
                                     ident[0:H, 0:H], start=True, stop=True)
                nc.vector.tensor_copy(d["ncs_nat"][:, 4 * q : 4 * q + 4, :],
                                      tpn[:])
                for k in range(4):
                    j = 4 * q + k
                    for n in range(P // NKC):
                        nc.tensor.matmul(
                            pmT[n * H : (n + 1) * H, :],
                            d["ncs_nat"][:, j, :],
                            d["slabT"][j][:, n * NKC : (n + 1) * NKC],
                            start=(j == 0), stop=(j == NC - 1),
                            tile_position=(0, n * H), skip_group_check=True)

            d["ncs_nat"] = st_pool.tile([PB, NC, H], BF16, tag="ncs_nat",
                                        name="ncs_nat")
            yield from _gru_gen(tc, gru_pool, ps_misc, wT["ihc"], wT["hhc"],
                                d["csT"], d["cmT_s"], bias["c"][0],
                                bias["c"][1], st_pool, "mid", hold,
                                dt_b=BF16, follow=follow_c)
            ncsT = hold["out"]
            if "ncsT" in dbg:
                nc.scalar.dma_start(dbg["ncsT"][b], ncsT[:])
            _pack_out(tc, ncsT, out_nc[b], ps_misc, out_pool, idents)
            yield
            pmT_s = st_pool.tile([H, P], F32, tag="mid", name="pmT_s")
            for n in range(4):
                nc.scalar.copy(pmT_s[:, n * NKC : (n + 1) * NKC],
                               pmT[n * H : (n + 1) * H, :])
            if "pmT" in dbg:
                nc.scalar.dma_start(dbg["pmT"][b], pmT_s[:])
            yield
            # ---- GRU-p -> packed output, pipelined per 512-chunk ----
            hold_p = {}
            sb = out_pool.tile([PB, 4, 4, H], F32, tag="opack", name="sb")

            def follow_p(q):
                npT = hold_p["out"]
                src_r = npT.rearrange("h (g q l) -> h g q l", g=4, l=4)
                pk = ps_misc.tile([PB, 4, H], F32, tag="sm", name="pk")
                for l in range(4):
                    nc.tensor.matmul(pk[:, l, :], src_r[:, q, :, l],
                                     ident_f[0:H, 0:H], start=True, stop=True)
                nc.scalar.copy(sb[:, q, :, :], pk[:])

            yield from _gru_gen(tc, gru_pool, ps_misc, wT["ihp"], wT["hhp"],
                                d["psT"], pmT_s, bias["p"][0], bias["p"][1],
                                st_pool, "hback", hold_p, dt_b=F32,
                                follow=follow_p)
            nc.scalar.dma_start(
                out_np[b].rearrange("(g q l) h -> q g l h", q=PB, l=4), sb[:])

        # ================= main schedule =================
        tail = iter(())

        def drain(n):
            for _ in range(n):
                next(tail, None)

        emit_states_dma(0)
        for b in range(BPC):
            emit_head(b)
            for i in range(NP):
                emit_slab(b, i)
                if i == 7 and b + 1 < BPC:
                    emit_states_dma(b + 1)
                drain(2)
            for _ in tail:
                pass
            emit_cm_extract(b)
            emit_at_prefetch(b)
            tail = tail_gen(b)
        for _ in tail:
            pass
def _pack_out(tc, srcT, dram_b, ps_misc, out_pool, idents):
    """srcT [H, N] -> HBM [N, H] f32 with 512B-per-partition runs."""
    nc = tc.nc
    dt = srcT.dtype
    N = srcT.shape[-1]
    NG = N // 512
    src_r = srcT.rearrange("h (g q l) -> h g q l", g=NG, l=4)
    sb = out_pool.tile([PB, NG, 4, H], F32, tag="opack", name="sb")
    for g in range(NG):
        pk = ps_misc.tile([PB, 4, H], F32, tag="sm", name="pk")
        for l in range(4):
            nc.tensor.matmul(pk[:, l, :], src_r[:, g, :, l],
                             idents[dt][0:H, 0:H], start=True, stop=True)
        nc.scalar.copy(sb[:, g, :, :], pk[:])
    nc.scalar.dma_start(
        dram_b.rearrange("(g q l) h -> q g l h", q=PB, l=4), sb[:])


# ---------------------------------------------------------------------------
# host-side entry
# ---------------------------------------------------------------------------

_NC_CACHE = {}


def _get_nc(debug_outputs=False):
    key = bool(debug_outputs)
    if key not in _NC_CACHE:
        _NC_CACHE[key] = build_nc(debug_outputs=key)
    return _NC_CACHE[key]


def kernel(path_states, channel_states, adj_matrix,
           w_ih_c, w_hh_c, b_ih_c, b_hh_c,
           w_ih_p, w_hh_p, b_ih_p, b_hh_p,
           _debug=False, _trace=False):
    nc = _get_nc(debug_outputs=_debug)
    f32 = np.float32
    adj16 = np.ascontiguousarray(np.asarray(adj_matrix, np.float16))
    adjT16 = np.ascontiguousarray(adj16.transpose(0, 2, 1))
    in_maps = []
    for k in range(NCORES):
        s = slice(k * BPC, (k + 1) * BPC)
        in_maps.append({
            "adj16": adj16[s],
            "adjT16": adjT16[s],
            "ps": np.ascontiguousarray(path_states[s], f32),
            "cs": np.ascontiguousarray(channel_states[s], f32),
            "w_ih_c": np.ascontiguousarray(w_ih_c, f32),
            "w_hh_c": np.ascontiguousarray(w_hh_c, f32),
            "w_ih_p": np.ascontiguousarray(w_ih_p, f32),
            "w_hh_p": np.ascontiguousarray(w_hh_p, f32),
            "b_ih_c": np.ascontiguousarray(b_ih_c, f32).reshape(G, 1),
            "b_hh_c": np.ascontiguousarray(b_hh_c, f32).reshape(G, 1),
            "b_ih_p": np.ascontiguousarray(b_ih_p, f32).reshape(G, 1),
            "b_hh_p": np.ascontiguousarray(b_hh_p, f32).reshape(G, 1),
        })
    res = run_bass_kernel_spmd(nc, in_maps, core_ids=list(range(NCORES)),
                               trace=_trace)
    new_path = np.concatenate([res.results[k]["new_path"] for k in range(NCORES)])
    new_channel = np.concatenate(
        [res.results[k]["new_channel"] for k in range(NCORES)])
    out = (new_path, new_channel)
    if _debug or _trace:
        return out, res
    return out



# revision 24
# speedup vs baseline: 1.9544x; 1.9544x over previous
"""Trainium2 Bass kernel for nn_MessagePassingLayer (GNN message passing).

reference semantics (per batch b):
  cm  = adj[b].T @ ps[b]                  # [C, H] channel aggregation
  ncs = GRUCell(x=cs[b], h=cm)            # new channel states
  pm  = adj[b] @ ncs                      # [P, H] path aggregation
  nps = GRUCell(x=ps[b], h=pm)            # new path states
  returns (nps, ncs)

Sharding: data-parallel over batch, 2 batches per core x 8 cores.

v2 design (trace-driven rework of v1):
  - einsum1/2 unchanged: adj p-slabs / adjT c-slabs as matmul moving data,
    cmT/pmT accumulate column-PACKED in one PSUM bank (rows 32n+h = token
    chunk n) via 4 concurrent col-group matmuls per slab.
  - GRU gates land in 4 per-TYPE PSUM banks with the same 32-row packing,
    so every eltwise op is a full-lane [128, 512] op (v1 used [32, 512]
    feature-major ops = 75% of DVE/ACT lanes idle, and its chained
    per-chunk GRU tail serialized ~148us of the kernel).
  - One matmul per (gate-type, chunk): stationary [65, 32] = w_ih^T over
    xT rows 0:32, w_hh^T over hT rows 32:64, bias on ones-row 64. Host
    pre-builds the combined weights; bias adds disappear.
  - h (= cm / pm) is read straight from the packed PSUM accumulator by
    the (h - n) op; only the SBUF copy for the gate matmuls' moving
    operand remains.
  - Host pre-transposes states (xT fp16) and pre-packs ps into the
    [128, 16, 32] natural-block layout -> all state DMAs are contiguous.
  - Outputs leave packed [128, 512] f32 (2KB/partition contiguous DMA);
    host unpacks with a free numpy transpose.
  - adjT ships as 2 x 4MB DMAs per batch (v1: 16 x 512KB SWDGE).
"""

import numpy as np

import concourse.bass as bass
import concourse.tile as tile
from concourse import bacc, masks, mybir
from concourse.bass_utils import run_bass_kernel_spmd

F32 = mybir.dt.float32
F16 = mybir.dt.float16

B, P, C, H = 16, 2048, 2048, 32
G = 3 * H  # 96
NCORES = 8
BPC = B // NCORES  # batches per core
PB = 128  # partition block
NP = P // PB  # 16 p-chunks
NC = C // PB  # 16 c-chunks
NKC = 512  # matmul moving chunk (one PSUM f32 bank)
KX = 2 * H + 1  # 65: xT rows 0:32, hT rows 32:64, ones row 64


def _gru_gen(nc, gpool, ps_g, h_psum, mm_emit, out32, out16, dt_big):
    """Packed full-lane GRUCell.

    h_psum:  [128, 512] f32 PSUM, rows 32n+h = hidden h for token chunk n
    mm_emit: mm_emit(gb, t, n) emits the gate matmul(s) for type t, chunk n
             into gb[t] rows 32n (types: 0=r, 1=z, 2=i_n, 3=h_n)
    out32:   [128, 512] f32 SBUF packed new state
    out16:   optional [128, 512] fp16 SBUF copy of out32
    dt_big:  dtype for intermediates that can reach |h| magnitude
    """
    AF = mybir.ActivationFunctionType
    gb = [ps_g.tile([PB, NKC], F32, tag=f"g{t}", name=f"g{t}") for t in range(4)]
    for t in range(4):
        for n in range(4):
            mm_emit(gb, t, n)
        yield
    r = gpool.tile([PB, NKC], F16, tag="gru_r", name="r")
    z = gpool.tile([PB, NKC], F16, tag="gru_z", name="z")
    t1 = gpool.tile([PB, NKC], dt_big, tag="gru_t1", name="t1")
    g2 = gpool.tile([PB, NKC], dt_big, tag="gru_g2", name="g2")
    n_ = gpool.tile([PB, NKC], F16, tag="gru_n", name="n_")
    d_ = gpool.tile([PB, NKC], F32, tag="gru_d", name="d_")
    nc.scalar.activation(r[:], gb[0][:], AF.Sigmoid)
    yield
    nc.scalar.activation(z[:], gb[1][:], AF.Sigmoid)
    yield
    nc.vector.tensor_mul(t1[:], r[:], gb[3][:])  # r * h_n
    yield
    nc.vector.tensor_add(g2[:], gb[2][:], t1[:])  # i_n + r*h_n
    yield
    nc.scalar.activation(n_[:], g2[:], AF.Tanh)
    yield
    nc.vector.tensor_sub(d_[:], h_psum[:], n_[:])  # h - n
    yield
    nc.vector.tensor_mul(t1[:], z[:], d_[:])  # z*(h-n)
    yield
    nc.vector.tensor_add(out32[:], n_[:], t1[:])  # n + z*(h-n)
    yield
    if out16 is not None:
        nc.scalar.copy(out16[:], out32[:])
        yield


def build_nc(debug_outputs=False, n_devices=NCORES):
    nc = bacc.Bacc("TRN2", target_bir_lowering=False, debug=False,
                   num_devices=n_devices)

    adj = nc.dram_tensor("adj16", [BPC, P, C], F16, kind="ExternalInput")
    adjT = nc.dram_tensor("adjT16", [BPC, C, P], F16, kind="ExternalInput")
    psL = nc.dram_tensor("psL16", [BPC, PB, NP, H], F16, kind="ExternalInput")
    xTp = nc.dram_tensor("xTp16", [BPC, H, P], F16, kind="ExternalInput")
    xTc = nc.dram_tensor("xTc16", [BPC, H, C], F16, kind="ExternalInput")
    wcb_c = nc.dram_tensor("wcb_c", [KX, 4 * H], F16, kind="ExternalInput")
    wpx = nc.dram_tensor("wpx", [H + 1, 4 * H], F16, kind="ExternalInput")
    wph = nc.dram_tensor("wph", [H + 1, 4 * H], F32, kind="ExternalInput")
    out_nc = nc.dram_tensor("nc_pk", [BPC, PB, NKC], F32, kind="ExternalOutput")
    out_np = nc.dram_tensor("np_pk", [BPC, PB, NKC], F32, kind="ExternalOutput")
    dbg = {}
    if debug_outputs:
        dbg["cmT"] = nc.dram_tensor("dbg_cmT", [BPC, H, C], F16, kind="ExternalOutput")
        dbg["pmT"] = nc.dram_tensor("dbg_pmT", [BPC, H, P], F32, kind="ExternalOutput")
        dbg["ncs"] = nc.dram_tensor("dbg_ncs", [BPC, PB, NKC], F16, kind="ExternalOutput")

    with tile.TileContext(nc) as tc:
        _body(tc, adj, adjT, psL, xTp, xTc, wcb_c, (wpx, wph), out_np, out_nc, dbg)
    nc.finalize()
    return nc


def _body(tc, adj, adjT, psL, xTp, xTc, wcb_c, wcb_p2, out_np, out_nc, dbg):
    nc = tc.nc
    from contextlib import ExitStack

    ctx = ExitStack()
    with ctx:
        const = ctx.enter_context(tc.tile_pool(name="const", bufs=1))
        a_pool = ctx.enter_context(tc.tile_pool(name="a_slabs", bufs=6))
        at_pool = ctx.enter_context(tc.tile_pool(name="at_slabs", bufs=2))
        st_pool = ctx.enter_context(tc.tile_pool(name="states", bufs=2))
        gpool = ctx.enter_context(tc.tile_pool(name="gru", bufs=1))
        out_pool = ctx.enter_context(tc.tile_pool(name="outs", bufs=2))
        ps_mm = ctx.enter_context(tc.tile_pool(name="ps_mm", bufs=3, space="PSUM"))
        ps_g = ctx.enter_context(tc.tile_pool(name="ps_g", bufs=1, space="PSUM"))
        ps_misc = ctx.enter_context(tc.tile_pool(name="ps_misc", bufs=1, space="PSUM"))

        # full identity; diagonal 32x32 blocks give an I32 moving operand at
        # any base partition 32q (to match a stationary sliced at 32q).
        ident = const.tile([PB, PB], F16)
        masks.make_identity(nc, ident[:])
        wcb = {}
        for s, wdram, kx, dt in (("c", wcb_c, KX, F16),
                                 ("px", wcb_p2[0], H + 1, F16),
                                 ("ph", wcb_p2[1], H + 1, F32)):
            w = const.tile([kx, 4 * H], dt, tag=f"wcb_{s}", name="w")
            nc.gpsimd.dma_start(w[:], wdram[:, :])
            wcb[s] = w

        state = [dict() for _ in range(BPC)]

        def emit_states_dma(b):
            d = state[b]
            d["psL"] = st_pool.tile([PB, NP, H], F16, tag="psL", name="psL")
            nc.gpsimd.dma_start(d["psL"][:], psL[b])
            # GRU-c moving operand: rows 0:32 csT, 32:64 cmT (extract), 64 ones
            xh = st_pool.tile([KX, C], F16, tag="xh_c", name="xh")
            nc.gpsimd.dma_start(xh[0:H, :], xTc[b])
            nc.gpsimd.memset(xh[2 * H : KX, :], 1.0)
            d["xh_c"] = xh
            # GRU-p x-side moving: rows 0:32 psT, 32 ones (fp16)
            xp = st_pool.tile([H + 1, P], F16, tag="xh_p", name="xp")
            nc.gpsimd.dma_start(xp[0:H, :], xTp[b])
            nc.gpsimd.memset(xp[H : H + 1, :], 1.0)
            d["xh_p"] = xp
            # GRU-p h-side moving: rows 0:32 pm (extract), 32 ones (f32)
            pm32 = st_pool.tile([H + 1, P], F32, tag="pm32", name="pm32")
            nc.gpsimd.memset(pm32[H : H + 1, :], 1.0)
            d["pm32"] = pm32

        def emit_head(b):
            state[b]["cmT"] = ps_mm.tile([PB, NKC], F32, tag="mm", name="cmT")

        def emit_slab(b, i):
            # einsum1: adj p-slab moving; cmT packed one bank (rows 32n).
            d = state[b]
            slab = a_pool.tile([PB, C], F16, tag="a", name="slab")
            nc.sync.dma_start(slab[:], adj[b, i * PB : (i + 1) * PB, :])
            for n in range(C // NKC):
                nc.tensor.matmul(
                    d["cmT"][n * H : (n + 1) * H, :],
                    d["psL"][:, i, :],
                    slab[:, n * NKC : (n + 1) * NKC],
                    start=(i == 0), stop=(i == NP - 1),
                    tile_position=(0, n * H), skip_group_check=True)

        def emit_at(b, hf):
            d = state[b]
            at = at_pool.tile([PB, NC // 2, P], F16, tag="at", name="at")
            nc.scalar.dma_start(
                at[:],
                adjT[b, hf * (C // 2) : (hf + 1) * (C // 2), :].rearrange(
                    "(j c) p -> c j p", c=PB))
            d.setdefault("at", []).append(at)

        def tail_gen(b):
            d = state[b]
            xh_c, xh_p, pm32 = d["xh_c"], d["xh_p"], d["pm32"]
            # ---- extract cmT -> xh_c rows 32:64 (fp16, gate moving) ----
            for n in range(4):
                eng = nc.vector.tensor_copy if n % 2 else nc.scalar.copy
                eng(xh_c[H : 2 * H, n * NKC : (n + 1) * NKC],
                    d["cmT"][n * H : (n + 1) * H, :])
                yield
            if "cmT" in dbg:
                nc.gpsimd.dma_start(dbg["cmT"][b], xh_c[H : 2 * H, :])

            # ---- GRU-c ----
            def mm_c(gb, t, n):
                nc.tensor.matmul(gb[t][n * H : (n + 1) * H, :],
                                 wcb["c"][:, t * H : (t + 1) * H],
                                 xh_c[:, n * NKC : (n + 1) * NKC],
                                 start=True, stop=True,
                                 tile_position=(0, n * H), skip_group_check=True)

            ncs32 = out_pool.tile([PB, NKC], F32, tag="ncs32", name="ncs32")
            ncs16 = out_pool.tile([PB, NKC], F16, tag="ncs16", name="ncs16")
            yield from _gru_gen(nc, gpool, ps_g, d["cmT"], mm_c,
                                ncs32, ncs16, F16)
            nc.scalar.dma_start(out_nc[b], ncs32[:])
            if "ncs" in dbg:
                nc.gpsimd.dma_start(dbg["ncs"][b], ncs16[:])
            # ---- backT: packed ncs16 -> natural [128, 16, 32] ----
            ncs_nat = st_pool.tile([PB, NC, H], F16, tag="ncs_nat",
                                   name="ncs_nat", bufs=1)
            for q in range(4):
                tpn = ps_misc.tile([PB, 4, H], F32, tag="sm", name="tpn")
                for k in range(4):
                    nc.tensor.matmul(
                        tpn[:, k, :],
                        ncs16[q * H : (q + 1) * H, k * PB : (k + 1) * PB],
                        ident[q * H : (q + 1) * H, q * H : (q + 1) * H],
                        start=True, stop=True, tile_position=(q * H, 0))
                nc.vector.tensor_copy(ncs_nat[:, 4 * q : 4 * q + 4, :], tpn[:])
                yield
            # ---- einsum2: adjT c-slabs moving; pmT packed ----
            pmT = ps_mm.tile([PB, NKC], F32, tag="mm", name="pmT")
            for j in range(NC):
                at = d["at"][j // (NC // 2)]
                jj = j % (NC // 2)
                for n in range(P // NKC):
                    nc.tensor.matmul(
                        pmT[n * H : (n + 1) * H, :],
                        ncs_nat[:, j, :],
                        at[:, jj, n * NKC : (n + 1) * NKC],
                        start=(j == 0), stop=(j == NC - 1),
                        tile_position=(0, n * H), skip_group_check=True)
                if j % 2:
                    yield
            # ---- extract pmT -> pm32 rows 0:32 (f32, h-side gate moving) ----
            for n in range(4):
                eng = nc.vector.tensor_copy if n % 2 else nc.scalar.copy
                eng(pm32[0:H, n * NKC : (n + 1) * NKC],
                    pmT[n * H : (n + 1) * H, :])
                yield
            if "pmT" in dbg:
                nc.gpsimd.dma_start(dbg["pmT"][b], pm32[0:H, :])

            # ---- GRU-p: fp16 x-side MM + f32 h-side MM ----
            def mm_p(gb, t, n):
                # t: 0=r, 1=z (x+h), 2=i_n (x only), 3=h_n (h only)
                sl = slice(n * NKC, (n + 1) * NKC)
                dst = gb[t][n * H : (n + 1) * H, :]
                wsl = slice(t * H, (t + 1) * H)
                if t != 3:
                    nc.tensor.matmul(dst, wcb["px"][:, wsl], xh_p[:, sl],
                                     start=True, stop=(t == 2),
                                     tile_position=(0, n * H),
                                     skip_group_check=True)
                if t != 2:
                    nc.tensor.matmul(dst, wcb["ph"][:, wsl], pm32[:, sl],
                                     start=(t == 3), stop=True,
                                     tile_position=(0, n * H),
                                     skip_group_check=True)

            nps32 = out_pool.tile([PB, NKC], F32, tag="nps32", name="nps32")
            yield from _gru_gen(nc, gpool, ps_g, pmT, mm_p,
                                nps32, None, F32)
            nc.scalar.dma_start(out_np[b], nps32[:])

        # ================= main schedule =================
        tail = iter(())

        def drain(k):
            for _ in range(k):
                next(tail, None)

        emit_states_dma(0)
        for b in range(BPC):
            emit_head(b)
            for i in range(NP):
                emit_slab(b, i)
                if i == 0:
                    emit_at(b, 0)
                    emit_at(b, 1)
                if i == 7 and b + 1 < BPC:
                    emit_states_dma(b + 1)
                drain(3)
            for _ in tail:
                pass
            tail = tail_gen(b)
        for _ in tail:
            pass


# ---------------------------------------------------------------------------
# host-side entry
# ---------------------------------------------------------------------------

_NC_CACHE = {}


def _get_nc(debug_outputs=False):
    key = bool(debug_outputs)
    if key not in _NC_CACHE:
        _NC_CACHE[key] = build_nc(debug_outputs=key)
    return _NC_CACHE[key]


def _build_wcb(w_ih, w_hh, b_ih, b_hh):
    """[65, 128] combined gate weights: cols 32t = type (r, z, in, hn)."""
    wcb = np.zeros((KX, 4 * H), np.float32)
    w_ih = np.asarray(w_ih, np.float32)
    w_hh = np.asarray(w_hh, np.float32)
    b_ih = np.asarray(b_ih, np.float32)
    b_hh = np.asarray(b_hh, np.float32)
    wcb[0:H, 0:H] = w_ih[0:H].T
    wcb[0:H, H : 2 * H] = w_ih[H : 2 * H].T
    wcb[0:H, 2 * H : 3 * H] = w_ih[2 * H : G].T
    wcb[H : 2 * H, 0:H] = w_hh[0:H].T
    wcb[H : 2 * H, H : 2 * H] = w_hh[H : 2 * H].T
    wcb[H : 2 * H, 3 * H : 4 * H] = w_hh[2 * H : G].T
    wcb[2 * H, 0:H] = b_ih[0:H] + b_hh[0:H]
    wcb[2 * H, H : 2 * H] = b_ih[H : 2 * H] + b_hh[H : 2 * H]
    wcb[2 * H, 2 * H : 3 * H] = b_ih[2 * H : G]
    wcb[2 * H, 3 * H : 4 * H] = b_hh[2 * H : G]
    return np.ascontiguousarray(wcb, np.float16)


def _build_wp(w_ih, w_hh, b_ih, b_hh):
    """GRU-p split weights: wpx [33, 128] fp16 (x side), wph [33, 128] f32
    (h side) — the h-side matmul runs in f32 because pm is large and the
    r/z presums cancel catastrophically at fp16 weight precision."""
    w_ih = np.asarray(w_ih, np.float32)
    w_hh = np.asarray(w_hh, np.float32)
    b_ih = np.asarray(b_ih, np.float32)
    b_hh = np.asarray(b_hh, np.float32)
    wpx = np.zeros((H + 1, 4 * H), np.float32)
    wph = np.zeros((H + 1, 4 * H), np.float32)
    wpx[0:H, 0:H] = w_ih[0:H].T
    wpx[0:H, H : 2 * H] = w_ih[H : 2 * H].T
    wpx[0:H, 2 * H : 3 * H] = w_ih[2 * H : G].T
    wpx[H, 0:H] = b_ih[0:H]
    wpx[H, H : 2 * H] = b_ih[H : 2 * H]
    wpx[H, 2 * H : 3 * H] = b_ih[2 * H : G]
    wph[0:H, 0:H] = w_hh[0:H].T
    wph[0:H, H : 2 * H] = w_hh[H : 2 * H].T
    wph[0:H, 3 * H : 4 * H] = w_hh[2 * H : G].T
    wph[H, 0:H] = b_hh[0:H]
    wph[H, H : 2 * H] = b_hh[H : 2 * H]
    wph[H, 3 * H : 4 * H] = b_hh[2 * H : G]
    return (np.ascontiguousarray(wpx, np.float16),
            np.ascontiguousarray(wph, np.float32))


def _unpack(pk):
    """[BPC, 128, 512] packed (rows 32n+h, cols j) -> [BPC, 2048, 32]."""
    return np.ascontiguousarray(
        pk.reshape(BPC, 4, H, NKC).transpose(0, 1, 3, 2).reshape(BPC, C, H))


def kernel(path_states, channel_states, adj_matrix,
           w_ih_c, w_hh_c, b_ih_c, b_hh_c,
           w_ih_p, w_hh_p, b_ih_p, b_hh_p,
           _debug=False, _trace=False):
    nc = _get_nc(debug_outputs=_debug)
    f16 = np.float16
    adj16 = np.ascontiguousarray(np.asarray(adj_matrix, f16))
    adjT16 = np.ascontiguousarray(adj16.transpose(0, 2, 1))
    ps16 = np.asarray(path_states, f16)
    cs16 = np.asarray(channel_states, f16)
    wc = _build_wcb(w_ih_c, w_hh_c, b_ih_c, b_hh_c)
    wpx, wph = _build_wp(w_ih_p, w_hh_p, b_ih_p, b_hh_p)
    in_maps = []
    for k in range(NCORES):
        s = slice(k * BPC, (k + 1) * BPC)
        psk, csk = ps16[s], cs16[s]
        in_maps.append({
            "adj16": adj16[s],
            "adjT16": adjT16[s],
            "psL16": np.ascontiguousarray(
                psk.reshape(BPC, NP, PB, H).transpose(0, 2, 1, 3)),
            "xTp16": np.ascontiguousarray(psk.transpose(0, 2, 1)),
            "xTc16": np.ascontiguousarray(csk.transpose(0, 2, 1)),
            "wcb_c": wc,
            "wpx": wpx,
            "wph": wph,
        })
    res = run_bass_kernel_spmd(nc, in_maps, core_ids=list(range(NCORES)),
                               trace=_trace)
    new_path = np.concatenate(
        [_unpack(res.results[k]["np_pk"]) for k in range(NCORES)])
    new_channel = np.concatenate(
        [_unpack(res.results[k]["nc_pk"]) for k in range(NCORES)])
    out = (new_path, new_channel)
    if _debug or _trace:
        return out, res
    return out


# revision 29
# speedup vs baseline: 2.0255x; 1.0364x over previous
"""Trainium2 Bass kernel for nn_MessagePassingLayer (GNN message passing).

reference semantics (per batch b):
  cm  = adj[b].T @ ps[b]                  # [C, H] channel aggregation
  ncs = GRUCell(x=cs[b], h=cm)            # new channel states
  pm  = adj[b] @ ncs                      # [P, H] path aggregation
  nps = GRUCell(x=ps[b], h=pm)            # new path states
  returns (nps, ncs)

Sharding: data-parallel over batch, 2 batches per core x 8 cores.

v2 design (trace-driven rework of v1):
  - einsum1/2 unchanged: adj p-slabs / adjT c-slabs as matmul moving data,
    cmT/pmT accumulate column-PACKED in one PSUM bank (rows 32n+h = token
    chunk n) via 4 concurrent col-group matmuls per slab.
  - GRU gates land in 4 per-TYPE PSUM banks with the same 32-row packing,
    so every eltwise op is a full-lane [128, 512] op (v1 used [32, 512]
    feature-major ops = 75% of DVE/ACT lanes idle, and its chained
    per-chunk GRU tail serialized ~148us of the kernel).
  - One matmul per (gate-type, chunk): stationary [65, 32] = w_ih^T over
    xT rows 0:32, w_hh^T over hT rows 32:64, bias on ones-row 64. Host
    pre-builds the combined weights; bias adds disappear.
  - h (= cm / pm) is read straight from the packed PSUM accumulator by
    the (h - n) op; only the SBUF copy for the gate matmuls' moving
    operand remains.
  - Host pre-transposes states (xT fp16) and pre-packs ps into the
    [128, 16, 32] natural-block layout -> all state DMAs are contiguous.
  - Outputs leave packed [128, 512] f32 (2KB/partition contiguous DMA);
    host unpacks with a free numpy transpose.
  - adjT ships as 2 x 4MB DMAs per batch (v1: 16 x 512KB SWDGE).
"""

import numpy as np

import concourse.bass as bass
import concourse.tile as tile
from concourse import bacc, masks, mybir
from concourse.bass_utils import run_bass_kernel_spmd

F32 = mybir.dt.float32
F16 = mybir.dt.float16

B, P, C, H = 16, 2048, 2048, 32
G = 3 * H  # 96
NCORES = 8
BPC = B // NCORES  # batches per core
PB = 128  # partition block
NP = P // PB  # 16 p-chunks
NC = C // PB  # 16 c-chunks
NKC = 512  # matmul moving chunk (one PSUM f32 bank)
KX = 2 * H + 1  # 65: xT rows 0:32, hT rows 32:64, ones row 64


def _gru_gen(nc, gpool, ps_g, h_psum, mm_emit, out32, out16, dt_big):
    """Packed full-lane GRUCell.

    h_psum:  [128, 512] f32 PSUM, rows 32n+h = hidden h for token chunk n
    mm_emit: mm_emit(gb, t, n) emits the gate matmul(s) for type t, chunk n
             into gb[t] rows 32n (types: 0=r, 1=z, 2=i_n, 3=h_n)
    out32:   [128, 512] f32 SBUF packed new state
    out16:   optional [128, 512] fp16 SBUF copy of out32
    dt_big:  dtype for intermediates that can reach |h| magnitude
    """
    AF = mybir.ActivationFunctionType
    gb = [ps_g.tile([PB, NKC], F32, tag=f"g{t}", name=f"g{t}") for t in range(4)]
    yield from mm_emit(gb)
    r = gpool.tile([PB, NKC], F16, tag="gru_r", name="r")
    z = gpool.tile([PB, NKC], F16, tag="gru_z", name="z")
    t1 = gpool.tile([PB, NKC], dt_big, tag="gru_t1", name="t1")
    g2 = gpool.tile([PB, NKC], dt_big, tag="gru_g2", name="g2")
    n_ = gpool.tile([PB, NKC], F16, tag="gru_n", name="n_")
    d_ = gpool.tile([PB, NKC], F32, tag="gru_d", name="d_")
    nc.scalar.activation(r[:], gb[0][:], AF.Sigmoid)
    yield
    nc.scalar.activation(z[:], gb[1][:], AF.Sigmoid)
    yield
    nc.vector.tensor_mul(t1[:], r[:], gb[3][:])  # r * h_n
    yield
    nc.vector.tensor_add(g2[:], gb[2][:], t1[:])  # i_n + r*h_n
    yield
    nc.scalar.activation(n_[:], g2[:], AF.Tanh)
    yield
    nc.vector.tensor_sub(d_[:], h_psum[:], n_[:])  # h - n
    yield
    nc.vector.tensor_mul(t1[:], z[:], d_[:])  # z*(h-n)
    yield
    nc.vector.tensor_add(out32[:], n_[:], t1[:])  # n + z*(h-n)
    yield
    if out16 is not None:
        nc.scalar.copy(out16[:], out32[:])
        yield


def build_nc(debug_outputs=False, n_devices=NCORES):
    nc = bacc.Bacc("TRN2", target_bir_lowering=False, debug=False,
                   num_devices=n_devices)

    adj = nc.dram_tensor("adj16", [BPC, P, C], F16, kind="ExternalInput")
    adjT = nc.dram_tensor("adjT16", [BPC, C, P], F16, kind="ExternalInput")
    psL = nc.dram_tensor("psL16", [BPC, PB, NP, H], F16, kind="ExternalInput")
    xTp = nc.dram_tensor("xTp16", [BPC, H, P], F16, kind="ExternalInput")
    xTc = nc.dram_tensor("xTc16", [BPC, H, C], F16, kind="ExternalInput")
    wcb_c = nc.dram_tensor("wcb_c", [KX, 4 * H], F16, kind="ExternalInput")
    wpx = nc.dram_tensor("wpx", [H + 1, 4 * H], F16, kind="ExternalInput")
    wph = nc.dram_tensor("wph", [H + 1, 4 * H], F32, kind="ExternalInput")
    out_nc = nc.dram_tensor("nc_pk", [BPC, PB, NKC], F32, kind="ExternalOutput")
    out_np = nc.dram_tensor("np_pk", [BPC, PB, NKC], F32, kind="ExternalOutput")
    dbg = {}
    if debug_outputs:
        dbg["cmT"] = nc.dram_tensor("dbg_cmT", [BPC, H, C], F16, kind="ExternalOutput")
        dbg["pmT"] = nc.dram_tensor("dbg_pmT", [BPC, H, P], F32, kind="ExternalOutput")
        dbg["ncs"] = nc.dram_tensor("dbg_ncs", [BPC, PB, NKC], F16, kind="ExternalOutput")

    with tile.TileContext(nc) as tc:
        _body(tc, adj, adjT, psL, xTp, xTc, wcb_c, (wpx, wph), out_np, out_nc, dbg)
    nc.finalize()
    return nc


def _body(tc, adj, adjT, psL, xTp, xTc, wcb_c, wcb_p2, out_np, out_nc, dbg):
    nc = tc.nc
    from contextlib import ExitStack

    ctx = ExitStack()
    with ctx:
        const = ctx.enter_context(tc.tile_pool(name="const", bufs=1))
        a_pool = ctx.enter_context(tc.tile_pool(name="a_slabs", bufs=6))
        at_pool = ctx.enter_context(tc.tile_pool(name="at_slabs", bufs=2))
        st_pool = ctx.enter_context(tc.tile_pool(name="states", bufs=2))
        gpool = ctx.enter_context(tc.tile_pool(name="gru", bufs=1))
        out_pool = ctx.enter_context(tc.tile_pool(name="outs", bufs=2))
        ps_mm = ctx.enter_context(tc.tile_pool(name="ps_mm", bufs=3, space="PSUM"))
        ps_g = ctx.enter_context(tc.tile_pool(name="ps_g", bufs=1, space="PSUM"))
        ps_misc = ctx.enter_context(tc.tile_pool(name="ps_misc", bufs=1, space="PSUM"))

        state = [dict() for _ in range(BPC)]
        ident = const.tile([PB, PB], F16)
        wcb = {}

        def emit_setup():
            # emitted after batch 0's input DMAs: identity and weights are
            # only needed by the (late) tail, so keep them off the queues'
            # critical path.
            # full identity; diagonal 32x32 blocks give an I32 moving
            # operand at any base partition 32q.
            masks.make_identity(nc, ident[:])
            for s, wdram, kx, dt in (("c", wcb_c, KX, F16),
                                     ("px", wcb_p2[0], H + 1, F16),
                                     ("ph", wcb_p2[1], H + 1, F32)):
                w = const.tile([kx, 4 * H], dt, tag=f"wcb_{s}", name="w")
                nc.gpsimd.dma_start(w[:], wdram[:, :])
                wcb[s] = w

        def emit_states_dma(b):
            d = state[b]
            d["psL"] = st_pool.tile([PB, NP, H], F16, tag="psL", name="psL")
            nc.gpsimd.dma_start(d["psL"][:], psL[b])
            # GRU-c moving operand: rows 0:32 csT, 32:64 cmT (extract), 64 ones
            xh = st_pool.tile([KX, C], F16, tag="xh_c", name="xh")
            nc.gpsimd.dma_start(xh[0:H, :], xTc[b])
            nc.gpsimd.memset(xh[2 * H : KX, :], 1.0)
            d["xh_c"] = xh
            # GRU-p x-side moving: rows 0:32 psT, 32 ones (fp16)
            xp = st_pool.tile([H + 1, P], F16, tag="xh_p", name="xp")
            nc.gpsimd.dma_start(xp[0:H, :], xTp[b])
            nc.gpsimd.memset(xp[H : H + 1, :], 1.0)
            d["xh_p"] = xp
            # GRU-p h-side moving: rows 0:32 pm (extract), 32 ones (f32)
            pm32 = st_pool.tile([H + 1, P], F32, tag="pm32", name="pm32")
            nc.gpsimd.memset(pm32[H : H + 1, :], 1.0)
            d["pm32"] = pm32

        def emit_head(b):
            state[b]["cmT"] = ps_mm.tile([PB, NKC], F32, tag="mm", name="cmT")

        def emit_slab(b, i):
            # einsum1: adj p-slab moving; cmT packed one bank (rows 32n).
            d = state[b]
            slab = a_pool.tile([PB, C], F16, tag="a", name="slab")
            nc.sync.dma_start(slab[:], adj[b, i * PB : (i + 1) * PB, :])
            for n in range(C // NKC):
                nc.tensor.matmul(
                    d["cmT"][n * H : (n + 1) * H, :],
                    d["psL"][:, i, :],
                    slab[:, n * NKC : (n + 1) * NKC],
                    start=(i == 0), stop=(i == NP - 1),
                    tile_position=(0, n * H), skip_group_check=True)

        def emit_at(b, hf):
            d = state[b]
            at = at_pool.tile([PB, NC // 2, P], F16, tag="at", name="at")
            nc.scalar.dma_start(
                at[:],
                adjT[b, hf * (C // 2) : (hf + 1) * (C // 2), :].rearrange(
                    "(j c) p -> c j p", c=PB))
            d.setdefault("at", []).append(at)

        def tail_gen(b):
            d = state[b]
            xh_c, xh_p, pm32 = d["xh_c"], d["xh_p"], d["pm32"]
            # ---- extract cmT -> xh_c rows 32:64 (fp16, gate moving) ----
            for n in range(4):
                eng = nc.vector.tensor_copy if n % 2 else nc.scalar.copy
                eng(xh_c[H : 2 * H, n * NKC : (n + 1) * NKC],
                    d["cmT"][n * H : (n + 1) * H, :])
                yield
            if "cmT" in dbg:
                nc.gpsimd.dma_start(dbg["cmT"][b], xh_c[H : 2 * H, :])

            # ---- GRU-c ----
            def mm_c(gb):
                for t in range(4):
                    for n in range(4):
                        nc.tensor.matmul(gb[t][n * H : (n + 1) * H, :],
                                         wcb["c"][:, t * H : (t + 1) * H],
                                         xh_c[:, n * NKC : (n + 1) * NKC],
                                         start=True, stop=True,
                                         tile_position=(0, n * H),
                                         skip_group_check=True)
                    yield

            ncs32 = out_pool.tile([PB, NKC], F32, tag="ncs32", name="ncs32")
            ncs16 = out_pool.tile([PB, NKC], F16, tag="ncs16", name="ncs16")
            yield from _gru_gen(nc, gpool, ps_g, d["cmT"], mm_c,
                                ncs32, ncs16, F16)
            nc.scalar.dma_start(out_nc[b], ncs32[:])
            if "ncs" in dbg:
                nc.gpsimd.dma_start(dbg["ncs"][b], ncs16[:])
            # ---- backT: packed ncs16 -> natural [128, 16, 32] ----
            ncs_nat = st_pool.tile([PB, NC, H], F16, tag="ncs_nat",
                                   name="ncs_nat", bufs=1)
            for q in range(4):
                tpn = ps_misc.tile([PB, 4, H], F32, tag="sm", name="tpn")
                for k in range(4):
                    nc.tensor.matmul(
                        tpn[:, k, :],
                        ncs16[q * H : (q + 1) * H, k * PB : (k + 1) * PB],
                        ident[q * H : (q + 1) * H, q * H : (q + 1) * H],
                        start=True, stop=True, tile_position=(q * H, 0))
                nc.vector.tensor_copy(ncs_nat[:, 4 * q : 4 * q + 4, :], tpn[:])
                yield
            # ---- einsum2: adjT c-slabs moving; pmT packed ----
            pmT = ps_mm.tile([PB, NKC], F32, tag="mm", name="pmT")
            for j in range(NC):
                at = d["at"][j // (NC // 2)]
                jj = j % (NC // 2)
                for n in range(P // NKC):
                    nc.tensor.matmul(
                        pmT[n * H : (n + 1) * H, :],
                        ncs_nat[:, j, :],
                        at[:, jj, n * NKC : (n + 1) * NKC],
                        start=(j == 0), stop=(j == NC - 1),
                        tile_position=(0, n * H), skip_group_check=True)
                if j % 2:
                    yield
            # ---- extract pmT -> pm32 rows 0:32 (f32, h-side gate moving) ----
            for n in range(4):
                eng = nc.vector.tensor_copy if n % 2 else nc.scalar.copy
                eng(pm32[0:H, n * NKC : (n + 1) * NKC],
                    pmT[n * H : (n + 1) * H, :])
                yield
            if "pmT" in dbg:
                nc.gpsimd.dma_start(dbg["pmT"][b], pm32[0:H, :])

            # ---- GRU-p: fp16 x-side MM + f32 h-side MM ----
            # x-side MMs emitted first: they don't depend on the pm extract,
            # so they overlap it. t: 0=r, 1=z (x+h), 2=i_n (x only),
            # 3=h_n (h only).
            def mm_p(gb):
                for t in range(3):
                    for n in range(4):
                        nc.tensor.matmul(
                            gb[t][n * H : (n + 1) * H, :],
                            wcb["px"][:, t * H : (t + 1) * H],
                            xh_p[:, n * NKC : (n + 1) * NKC],
                            start=True, stop=(t == 2),
                            tile_position=(0, n * H), skip_group_check=True)
                yield
                for t in (0, 1, 3):
                    for n in range(4):
                        nc.tensor.matmul(
                            gb[t][n * H : (n + 1) * H, :],
                            wcb["ph"][:, t * H : (t + 1) * H],
                            pm32[:, n * NKC : (n + 1) * NKC],
                            start=(t == 3), stop=True,
                            tile_position=(0, n * H), skip_group_check=True)
                    yield

            nps32 = out_pool.tile([PB, NKC], F32, tag="nps32", name="nps32")
            yield from _gru_gen(nc, gpool, ps_g, pmT, mm_p,
                                nps32, None, F32)
            nc.scalar.dma_start(out_np[b], nps32[:])

        # ================= main schedule =================
        tail = iter(())

        def drain(k):
            for _ in range(k):
                next(tail, None)

        emit_states_dma(0)
        for b in range(BPC):
            emit_head(b)
            for i in range(NP):
                emit_slab(b, i)
                if i == 0:
                    emit_at(b, 0)
                    emit_at(b, 1)
                    if b == 0:
                        emit_setup()
                if i == 7 and b + 1 < BPC:
                    emit_states_dma(b + 1)
                drain(3)
            for _ in tail:
                pass
            tail = tail_gen(b)
        for _ in tail:
            pass


# ---------------------------------------------------------------------------
# host-side entry
# ---------------------------------------------------------------------------

_NC_CACHE = {}


def _get_nc(debug_outputs=False):
    key = bool(debug_outputs)
    if key not in _NC_CACHE:
        _NC_CACHE[key] = build_nc(debug_outputs=key)
    return _NC_CACHE[key]


def _build_wcb(w_ih, w_hh, b_ih, b_hh):
    """[65, 128] combined gate weights: cols 32t = type (r, z, in, hn)."""
    wcb = np.zeros((KX, 4 * H), np.float32)
    w_ih = np.asarray(w_ih, np.float32)
    w_hh = np.asarray(w_hh, np.float32)
    b_ih = np.asarray(b_ih, np.float32)
    b_hh = np.asarray(b_hh, np.float32)
    wcb[0:H, 0:H] = w_ih[0:H].T
    wcb[0:H, H : 2 * H] = w_ih[H : 2 * H].T
    wcb[0:H, 2 * H : 3 * H] = w_ih[2 * H : G].T
    wcb[H : 2 * H, 0:H] = w_hh[0:H].T
    wcb[H : 2 * H, H : 2 * H] = w_hh[H : 2 * H].T
    wcb[H : 2 * H, 3 * H : 4 * H] = w_hh[2 * H : G].T
    wcb[2 * H, 0:H] = b_ih[0:H] + b_hh[0:H]
    wcb[2 * H, H : 2 * H] = b_ih[H : 2 * H] + b_hh[H : 2 * H]
    wcb[2 * H, 2 * H : 3 * H] = b_ih[2 * H : G]
    wcb[2 * H, 3 * H : 4 * H] = b_hh[2 * H : G]
    return np.ascontiguousarray(wcb, np.float16)


def _build_wp(w_ih, w_hh, b_ih, b_hh):
    """GRU-p split weights: wpx [33, 128] fp16 (x side), wph [33, 128] f32
    (h side) — the h-side matmul runs in f32 because pm is large and the
    r/z presums cancel catastrophically at fp16 weight precision."""
    w_ih = np.asarray(w_ih, np.float32)
    w_hh = np.asarray(w_hh, np.float32)
    b_ih = np.asarray(b_ih, np.float32)
    b_hh = np.asarray(b_hh, np.float32)
    wpx = np.zeros((H + 1, 4 * H), np.float32)
    wph = np.zeros((H + 1, 4 * H), np.float32)
    wpx[0:H, 0:H] = w_ih[0:H].T
    wpx[0:H, H : 2 * H] = w_ih[H : 2 * H].T
    wpx[0:H, 2 * H : 3 * H] = w_ih[2 * H : G].T
    wpx[H, 0:H] = b_ih[0:H]
    wpx[H, H : 2 * H] = b_ih[H : 2 * H]
    wpx[H, 2 * H : 3 * H] = b_ih[2 * H : G]
    wph[0:H, 0:H] = w_hh[0:H].T
    wph[0:H, H : 2 * H] = w_hh[H : 2 * H].T
    wph[0:H, 3 * H : 4 * H] = w_hh[2 * H : G].T
    wph[H, 0:H] = b_hh[0:H]
    wph[H, H : 2 * H] = b_hh[H : 2 * H]
    wph[H, 3 * H : 4 * H] = b_hh[2 * H : G]
    return (np.ascontiguousarray(wpx, np.float16),
            np.ascontiguousarray(wph, np.float32))


def _unpack(pk):
    """[BPC, 128, 512] packed (rows 32n+h, cols j) -> [BPC, 2048, 32]."""
    return np.ascontiguousarray(
        pk.reshape(BPC, 4, H, NKC).transpose(0, 1, 3, 2).reshape(BPC, C, H))


def kernel(path_states, channel_states, adj_matrix,
           w_ih_c, w_hh_c, b_ih_c, b_hh_c,
           w_ih_p, w_hh_p, b_ih_p, b_hh_p,
           _debug=False, _trace=False):
    nc = _get_nc(debug_outputs=_debug)
    f16 = np.float16
    adj16 = np.ascontiguousarray(np.asarray(adj_matrix, f16))
    adjT16 = np.ascontiguousarray(adj16.transpose(0, 2, 1))
    ps16 = np.asarray(path_states, f16)
    cs16 = np.asarray(channel_states, f16)
    wc = _build_wcb(w_ih_c, w_hh_c, b_ih_c, b_hh_c)
    wpx, wph = _build_wp(w_ih_p, w_hh_p, b_ih_p, b_hh_p)
    in_maps = []
    for k in range(NCORES):
        s = slice(k * BPC, (k + 1) * BPC)
        psk, csk = ps16[s], cs16[s]
        in_maps.append({
            "adj16": adj16[s],
            "adjT16": adjT16[s],
            "psL16": np.ascontiguousarray(
                psk.reshape(BPC, NP, PB, H).transpose(0, 2, 1, 3)),
            "xTp16": np.ascontiguousarray(psk.transpose(0, 2, 1)),
            "xTc16": np.ascontiguousarray(csk.transpose(0, 2, 1)),
            "wcb_c": wc,
            "wpx": wpx,
            "wph": wph,
        })
    res = run_bass_kernel_spmd(nc, in_maps, core_ids=list(range(NCORES)),
                               trace=_trace)
    new_path = np.concatenate(
        [_unpack(res.results[k]["np_pk"]) for k in range(NCORES)])
    new_channel = np.concatenate(
        [_unpack(res.results[k]["nc_pk"]) for k in range(NCORES)])
    out = (new_path, new_channel)
    if _debug or _trace:
        return out, res
    return out
